# revision 50
# baseline (speedup 1.0000x reference)
"""Trainium2 Bass kernel for a dense transformer attention block (v2).

Reference computation (per batch b, tokens n=2048, d=1024, 16 heads x 64):
    xn  = LayerNorm(x) * gamma + beta
    qkv = xn @ W_qkv^T ;  q,k,v per head
    att = softmax(q k^T / sqrt(hd)) v
    out = concat_heads(att) @ W_out^T

Sharding over 8 cores: data-parallel over the 4 batches x tensor-parallel over
2 head-groups of 8 heads.  Core c handles batch c//2, heads (c%2)*8 ..+8.
Each core produces a partial out^T (its heads' contribution); the host sums
the two partials per batch and transposes back.

v2 changes vs v1 (902978ns baseline -> ~496500ns measured by bench2):
  - final projection: 512-wide PSUM groups through the double-buffered
    mm pool with W_out preloaded during attention; kept as PURE TAIL -
    emitting it between pair-3 chunks stalls the in-order PE stream on
    the softmax-denominator DMA round trip (measured 604us vs 496us)
  - all attention operands in bf16 (q/k/v/P/o; 5.5e-3 rel err, gate 2e-2).
    HW-measured (mb_run.py): bf16 matmuls stream 2x faster than f32r
    (163 vs 339 ns for [128,128]x[128,512]; f32r pays 4-byte streaming)
  - S tiles are [128, 1024] PSUM (2 banks, both heads of a pair) so softmax
    exp is ONE 1024-wide ACT instruction per k-tile; the two S matmuls are
    row-tiled (tile_position (0,0)/(64,0) via base_partition)
  - attention output pairs stay in SBUF (bf16) - no DRAM round trip
  - softmax denominator: drain PSUM row -> DRAM -> partition-broadcast DMA
    back, OFF the critical path (oa PSUM freed by plain DVE copies; the
    normalize multiply runs on the otherwise-idle Pool engine)
  - LN normalize split across Pool (sub) and DVE (mul) engines
  - QKV projections emitted as 9-item "groups" (8 dc-matmuls + drain)
    with mm_ps bufs=2 so consecutive groups' accumulations overlap;
    V for pairs 0-1 / 2-3 woven per-k-tile into pair-0 chunks 0/1;
    q/k groups for later chunks sit at chunk boundaries (WEAVE=1
    spreads them 1-matmul-per-k-tile instead: measured SLOWER on HW)
  - PSUM budget: s2 2x2 + oa 2x1 + mm 2x1 = 8 banks exactly
  - x load / out store split across both HWDGE queues (SP + ACT)
"""

import numpy as np
import ml_dtypes

import concourse.bass as bass
import concourse.mybir as mybir
import concourse.tile as tile

P = 128
D = 1024            # model dim
NTOK = 2048         # tokens per batch
HD = 64             # head dim
NH = 16             # total heads
NH_CORE = 8         # heads per core
INNER_C = NH_CORE * HD   # 512 inner dims per core
DCH = D // P        # 8 d-chunks of 128
KT = NTOK // P      # 16 token tiles of 128 (attention k)
NQC = NTOK // 512   # 4 q-chunks of 512
LN_EPS = 1e-5

f32 = mybir.dt.float32
f32r = mybir.dt.float32r
bf16 = mybir.dt.bfloat16
AF = mybir.ActivationFunctionType

_WCTR = [0]

import os as _os
_PBUFS = int(_os.environ.get("P_BUFS", "6"))
_OUNBUFS = int(_os.environ.get("OUN_BUFS", "3"))
_DNBUFS = int(_os.environ.get("DN_BUFS", "3"))
_MWAITS = int(_os.environ.get("MAX_WAITS", "1"))
_EXPW = int(_os.environ.get("EXPW", "1024"))
# timing-lesion probes (produce wrong results; for bottleneck attribution)
_LESION = _os.environ.get("LESION", "")
_WEAVE = int(_os.environ.get("WEAVE", "0"))


def _legalize_waits(nc, max_waits=1):
    """Walrus wait-slot limits are tiny (fp32 matmul: 1). Hoist excess sync
    waits onto preceding same-engine NoOps - engines execute their stream in
    order, so this is semantics-preserving."""
    import bass_rust as _br
    for fn in nc.m.functions:
        for blk in fn.blocks:
            out = []
            for inst in blk.instructions:
                lim = 1
                if max_waits > 1 and isinstance(inst, mybir.InstActivation):
                    lim = max_waits
                si = getattr(inst, "sync_info", None)
                if si is not None and len(si.on_wait) > lim:
                    waits = list(si.on_wait)
                    keep, extra = waits[:lim], waits[lim:]
                    eng = inst.engine
                    for w in extra:
                        _WCTR[0] += 1
                        nop = mybir.InstNoOp(name=f"WNOP-{_WCTR[0]}",
                                             ins=[], outs=[])
                        nop.engine = eng
                        nop.sync_info = _br.SyncInfo(on_wait=[w], on_update=[])
                        out.append(nop)
                    inst.sync_info = _br.SyncInfo(on_wait=keep,
                                                  on_update=list(si.on_update))
                out.append(inst)
            blk.instructions[:] = out


def build_nc(loop_n=None, legalize=True):
    nc = bass.Bass()

    xT = nc.dram_tensor("xT", [D, NTOK], f32r, kind="ExternalInput")
    # [d, 1024]: cols 0:512 = q feats (8 heads x 64), cols 512:1024 = k feats
    wqkT = nc.dram_tensor("wqkT", [D, 2 * INNER_C], bf16, kind="ExternalInput")
    wvT = nc.dram_tensor("wvT", [D, INNER_C], bf16, kind="ExternalInput")
    woT = nc.dram_tensor("woT", [INNER_C, D], bf16, kind="ExternalInput")
    onesc = nc.dram_tensor("onesc", [P, P], f32r, kind="ExternalInput")
    vones = nc.dram_tensor("vones", [P, KT, 8, 1], bf16, kind="ExternalInput")
    # per-feature bias (W @ beta): col j<4 -> q pair j, col j>=4 -> k pair j-4
    cqk = nc.dram_tensor("cqk", [P, 8], f32, kind="ExternalInput")
    cv = nc.dram_tensor("cv", [1, INNER_C], f32, kind="ExternalInput")
    outT = nc.dram_tensor("outT", [D, NTOK], f32, kind="ExternalOutput")

    with tile.TileContext(nc) as tc:
        if loop_n:
            with tc.For_i(0, loop_n, 1):
                _emit(nc, tc, xT, wqkT, wvT, woT, cqk, cv, onesc, vones, outT)
        else:
            _emit(nc, tc, xT, wqkT, wvT, woT, cqk, cv, onesc, vones, outT)
    if legalize:
        _legalize_waits(nc, max_waits=_MWAITS)
    return nc


def _emit(nc, tc, xT, wqkT, wvT, woT, cqk, cv, onesc, vones, outT):
    from contextlib import ExitStack

    es = ExitStack()
    with es:
        const = es.enter_context(tc.tile_pool(name="const", bufs=1))
        ones_sb = const.tile([P, P], f32r)
        nc.sync.dma_start(ones_sb[:], onesc[:])
        cqk_sb = const.tile([P, 8], f32)
        nc.sync.dma_start(cqk_sb[:], cqk[:])
        cv_sb = const.tile([P, INNER_C], f32)
        nc.sync.dma_start(
            cv_sb[:],
            cv[0:1, :].partition_broadcast(P).rearrange("p o f -> p (o f)"))
        eps_sb = const.tile([P, 1], f32)
        nc.vector.memset(eps_sb[:], LN_EPS)

        # xhat (normalized x^T, bf16) persists through QKV projections;
        # o_pair (attention outputs, bf16) persists until Phase D
        xhat_pool = es.enter_context(tc.tile_pool(name="xhat", bufs=1))
        xhat = [xhat_pool.tile([P, NTOK], bf16, tag=f"xhat{dc}",
                               name=f"xhat{dc}")
                for dc in range(DCH)]
        o_pool = es.enter_context(tc.tile_pool(name="opair", bufs=1))
        o_pair = [o_pool.tile([P, NTOK], bf16, tag=f"o{pr}", name=f"o{pr}")
                  for pr in range(4)]
        wo_pool = es.enter_context(tc.tile_pool(name="wo", bufs=1))
        wo_sb = wo_pool.tile([P, 4, D], bf16)

        # ---------------- Phase A: LayerNorm in x^T layout ----------------
        with tc.tile_pool(name="xraw", bufs=1) as xraw_pool, \
             tc.tile_pool(name="lnps", bufs=1, space="PSUM") as lnps, \
             tc.tile_pool(name="lnsb", bufs=1) as lnsb, \
             tc.tile_pool(name="xsq", bufs=2) as xsq_pool, \
             tc.tile_pool(name="xmm", bufs=2) as xmm_pool:
            xraw = []
            for dc in range(DCH):
                t = xraw_pool.tile([P, NTOK], f32r, tag=f"xraw{dc}",
                                   name=f"xraw{dc}")
                # split the 8MB load across both HWDGE queues (ACT is idle)
                eng = nc.sync if dc % 2 == 0 else nc.scalar
                eng.dma_start(t[:], xT[dc * P:(dc + 1) * P, :])
                xraw.append(t)

            mu_ps = lnps.tile([P, NTOK], f32, tag="mu")
            sq_ps = lnps.tile([P, NTOK], f32, tag="sq")
            # replicated mean: ones(1/D) as stationary, x^T as moving
            for dc in range(DCH):
                for qc in range(NQC):
                    nc.tensor.matmul(
                        mu_ps[:, qc * 512:(qc + 1) * 512],
                        ones_sb[:],
                        xraw[dc][:, qc * 512:(qc + 1) * 512],
                        start=(dc == 0), stop=(dc == DCH - 1),
                    )
            for dc in range(DCH):
                sq = xsq_pool.tile([P, NTOK], f32r, tag="sq")
                nc.vector.tensor_mul(sq[:], xraw[dc][:].bitcast(f32),
                                     xraw[dc][:].bitcast(f32))
                for qc in range(NQC):
                    nc.tensor.matmul(
                        sq_ps[:, qc * 512:(qc + 1) * 512],
                        ones_sb[:],
                        sq[:, qc * 512:(qc + 1) * 512],
                        start=(dc == 0), stop=(dc == DCH - 1),
                    )

            mu_sb = lnsb.tile([P, NTOK], f32, tag="mu")
            rs_sb = lnsb.tile([P, NTOK], f32, tag="rs")
            var_sb = lnsb.tile([P, NTOK], f32, tag="var")
            nc.vector.tensor_copy(mu_sb[:], mu_ps[:])
            nc.vector.tensor_mul(var_sb[:], mu_sb[:], mu_sb[:])
            nc.vector.tensor_sub(var_sb[:], sq_ps[:], var_sb[:])
            # rstd = exp(-0.5 * ln(var + eps)); Ln/Exp share one ACT table set
            nc.scalar.activation(rs_sb[:], var_sb[:], AF.Ln, bias=eps_sb[:, :])
            nc.scalar.activation(rs_sb[:], rs_sb[:], AF.Exp, scale=-0.5)

            # normalize: sub on Pool, mul on DVE (parallel engines)
            for dc in range(DCH):
                xm = xmm_pool.tile([P, NTOK], f32, tag="xm")
                nc.gpsimd.tensor_sub(xm[:], xraw[dc][:].bitcast(f32), mu_sb[:])
                nc.vector.tensor_mul(xhat[dc][:], xm[:], rs_sb[:])

        # ------------- Phases B+C: QKV projection + attention -------------
        with tc.tile_pool(name="wqk", bufs=2) as wqk_pool, \
             tc.tile_pool(name="wv", bufs=1) as wv_pool, \
             tc.tile_pool(name="qkt", bufs=1) as qk_pool, \
             tc.tile_pool(name="vaug", bufs=1) as vaug_pool, \
             tc.tile_pool(name="mm_ps", bufs=2, space="PSUM") as mm_ps, \
             tc.tile_pool(name="s_ps", bufs=2, space="PSUM") as s_ps_pool, \
             tc.tile_pool(name="oa_ps", bufs=1, space="PSUM") as oa_ps_pool, \
             tc.tile_pool(name="p_sb", bufs=_PBUFS) as p_pool, \
             tc.tile_pool(name="oun", bufs=_OUNBUFS) as oun_pool, \
             tc.tile_pool(name="dn", bufs=_DNBUFS) as dn_pool, \
             tc.tile_pool(name="outsb", bufs=2) as out_pool, \
             tc.tile_pool(name="dnd", bufs=2, space="DRAM") as dnd_pool:
            # V in natural layout for all 8 heads, ones column per head
            vaug = vaug_pool.tile([P, KT, 8, HD + 1], bf16, tag="vaug")
            nc.scalar.dma_start(vaug[:, :, :, HD:HD + 1], vones[:])
            wv_sb = wv_pool.tile([P, DCH, INNER_C], bf16, tag="wv")
            nc.scalar.dma_start(
                wv_sb[:], wvT.rearrange("(dc p) f -> p dc f", p=P))
            nc.scalar.dma_start(
                wo_sb[:], woT.rearrange("(pc p) f -> p pc f", p=P))

            def wqk_dma_fns(pair):
                """Allocate the weight tile now; return the tile plus a
                deferred DMA emitter (so buffer reuse can be emitted AFTER
                the previous tenant's woven reads)."""
                wqk_sb = wqk_pool.tile([P, DCH, 256], bf16, tag="wqk",
                                       name=f"wqk{pair}")

                def dma():
                    nc.sync.dma_start(
                        wqk_sb[:, :, 0:128],
                        wqkT[:, pair * P:(pair + 1) * P]
                        .rearrange("(dc p) f -> p dc f", p=P))
                    nc.sync.dma_start(
                        wqk_sb[:, :, 128:256],
                        wqkT[:, 512 + pair * P:512 + (pair + 1) * P]
                        .rearrange("(dc p) f -> p dc f", p=P))

                return wqk_sb, dma

            # --- projection groups: lists of (pe_cost_ns, emit_fn) whose
            # matmuls get WOVEN one-at-a-time into the attention k-tile
            # loop so the PE never inserts a multi-us burst that starves
            # the ACT softmax pipeline.  mm_ps bufs=2 lets two groups'
            # accumulations overlap across the weave.
            def qk_group(pair, wqk_sb, kind, qc):
                st = {}

                def mk(dc):
                    def f():
                        if dc == 0:
                            st["ps"] = mm_ps.tile([P, 512], f32, tag="mm",
                                                  name=f"qk{pair}{kind}{qc}")
                        nc.tensor.matmul(
                            st["ps"][:],
                            wqk_sb[:, dc, kind * 128:kind * 128 + 128],
                            xhat[dc][:, qc * 512:(qc + 1) * 512],
                            start=(dc == 0), stop=(dc == DCH - 1),
                        )
                    return f

                def drain():
                    nc.vector.tensor_scalar_add(
                        qk_dst[pair % 2][kind][:, qc * 512:(qc + 1) * 512],
                        st["ps"][:],
                        cqk_sb[:, kind * 4 + pair:kind * 4 + pair + 1])

                if _LESION == "noproj":
                    return []
                return [(170, mk(dc)) for dc in range(DCH)] + [(10, drain)]

            def v_group(kt, fh):
                f0 = fh * 256
                st = {}

                def mk(dc):
                    def f():
                        if dc == 0:
                            st["ps"] = mm_ps.tile([P, 256], f32, tag="mm",
                                                  name=f"v{kt}{fh}",
                                                  padded_shape=[P, 512])
                        nc.tensor.matmul(
                            st["ps"][:],
                            xhat[dc][:, kt * P:(kt + 1) * P],
                            wv_sb[:, dc, f0:f0 + 256],
                            start=(dc == 0), stop=(dc == DCH - 1),
                        )
                    return f

                def drain():
                    nc.vector.tensor_add(
                        vaug[:, kt, fh * 4:fh * 4 + 4, 0:HD],
                        st["ps"][:].rearrange("p (h f) -> p h f", h=4),
                        cv_sb[:, f0:f0 + 256]
                        .rearrange("p (h f) -> p h f", h=4),
                    )

                if _LESION == "noproj":
                    return []
                return [(120, mk(dc)) for dc in range(DCH)] + [(10, drain)]

            fillers = []

            def pop_fillers(n_slots_left):
                # self-pacing: drain the queue evenly over remaining slots
                if not fillers:
                    return
                per = -(-len(fillers) // max(n_slots_left, 1))
                for _ in range(min(per, len(fillers))):
                    fillers.pop(0)[1]()

            slot_state = {"left": 0}

            def emit_attn_chunk(pair, qq, v_il=None, v_list=None):
                qt, kt_sb = qk_dst[pair % 2]
                oa = [oa_ps_pool.tile([HD + 1, 512], f32, tag=f"oa{hl}",
                                      name=f"oa{hl}")
                      for hl in range(2)]
                pts = {}
                for ktile in range(KT):
                    s2 = s_ps_pool.tile([P, 1024], f32, tag="s")
                    nhl = 1 if _LESION == "s" else 2
                    for hl in range(nhl):
                        hb = hl * HD
                        nc.tensor.matmul(
                            s2[:, hl * 512:(hl + 1) * 512],
                            kt_sb[hb:hb + HD, ktile * P:(ktile + 1) * P],
                            qt[hb:hb + HD, qq * 512:(qq + 1) * 512],
                            start=True, stop=True,
                        )
                    pt = p_pool.tile([P, 1024], bf16, tag="p")
                    if _LESION == "exp":
                        nc.scalar.activation(pt[:, 0:512], s2[:, 0:512], AF.Exp)
                    elif _EXPW == 1024:
                        nc.scalar.activation(pt[:], s2[:], AF.Exp)
                    else:
                        nc.scalar.activation(pt[:, 0:512], s2[:, 0:512], AF.Exp)
                        nc.scalar.activation(pt[:, 512:1024], s2[:, 512:1024],
                                             AF.Exp)
                    pts[ktile] = pt
                    if _WEAVE:
                        pop_fillers(slot_state["left"])
                    elif v_il is not None:
                        for _, fn in v_group(ktile, v_il):
                            fn()
                    elif v_list and ktile % 8 == 0:
                        for _, fn in v_group(v_list.pop(0), 1):
                            fn()
                    slot_state["left"] -= 1
                    if ktile >= 1:
                        _emit_pv(pair, oa, pts, ktile - 1)
                _emit_pv(pair, oa, pts, KT - 1)
                # drain: copies only (frees oa fast); normalize on Pool later
                o_un = oun_pool.tile([P, 512], f32, tag="oun")
                dnq = dn_pool.tile([1, 1024], f32, tag="dnq")
                for hl in range(2):
                    nc.vector.tensor_copy(
                        o_un[hl * HD:(hl + 1) * HD, :], oa[hl][0:HD, :])
                    nc.vector.tensor_copy(
                        dnq[0:1, hl * 512:(hl + 1) * 512],
                        oa[hl][HD:HD + 1, :])
                dscr = dnd_pool.tile([1, 1024], f32, tag="dscr")
                nc.sync.dma_start(dscr[:], dnq[:])
                rbc = dn_pool.tile([P, 512], f32, tag="rbc")
                for hl in range(2):
                    nc.sync.dma_start(
                        rbc[hl * HD:(hl + 1) * HD, :],
                        dscr[0:1, hl * 512:(hl + 1) * 512]
                        .partition_broadcast(HD)
                        .rearrange("p o f -> p (o f)"))
                nc.vector.reciprocal(rbc[:], rbc[:])
                nc.gpsimd.tensor_mul(
                    o_pair[pair][:, qq * 512:(qq + 1) * 512],
                    o_un[:], rbc[:])

            def _emit_pv(pair, oa, pts, ktile):
                pt = pts.pop(ktile)
                if _LESION == "pv" and ktile not in (0, KT - 1):
                    return
                for hl in range(2):
                    nc.tensor.matmul(
                        oa[hl][:],
                        vaug[:, ktile, 2 * pair + hl, :],
                        pt[:, hl * 512:(hl + 1) * 512],
                        start=(ktile == 0), stop=(ktile == KT - 1),
                    )

            # ---- emission schedule ----
            # Pre-attention head: pair-0 k (all), pair-0 q chunk 0, and V
            # features for pairs 0-1 (fh=0) - the minimum for chunk (0,0).
            # Everything else is woven into the k-tile slots, enqueued a
            # pair ahead of its consumer:
            #   pair-0 slots: q-p0 qc1-3, k-p1, q-p1 qc0, half of V fh=1
            #   pair-1 slots: q-p1 qc1-3, k-p2, q-p2 qc0, rest of V fh=1
            #   pair-2 slots: q-p2 qc1-3, k-p3, q-p3 qc0
            #   pair-3 slots: q-p3 qc1-3
            qk_dst = {}
            for pr2 in range(2):
                qt = qk_pool.tile([P, NTOK], bf16, tag=f"qt{pr2}",
                                  name=f"qt{pr2}")
                ktt = qk_pool.tile([P, NTOK], bf16, tag=f"kt{pr2}",
                                   name=f"kt{pr2}")
                qk_dst[pr2] = (qt, ktt)

            wqk = [None] * 4
            for pr in range(2):
                sb_, dma_ = wqk_dma_fns(pr)
                wqk[pr] = sb_
                dma_()
            for qc in range(4):
                for _, fn in qk_group(0, wqk[0], 1, qc):
                    fn()

            if _WEAVE:
                for _, fn in qk_group(0, wqk[0], 0, 0):
                    fn()
                for kt in range(KT):
                    for _, fn in v_group(kt, 0):
                        fn()
                for pair in range(4):
                    for qc in range(1, 4):
                        fillers.extend(qk_group(pair, wqk[pair], 0, qc))
                    if pair < 2:
                        sb_, dma_ = wqk_dma_fns(pair + 2)
                        wqk[pair + 2] = sb_
                        fillers.append((10, dma_))
                    if pair < 3:
                        for qc in range(4):
                            fillers.extend(
                                qk_group(pair + 1, wqk[pair + 1], 1, qc))
                        fillers.extend(qk_group(pair + 1, wqk[pair + 1], 0, 0))
                    if pair < 2:
                        for kt in range(pair * 8, pair * 8 + 8):
                            fillers.extend(v_group(kt, 1))
                    slot_state["left"] = KT * NQC
                    for qq in range(NQC):
                        emit_attn_chunk(pair, qq)
                while fillers:
                    fillers.pop(0)[1]()
                for qq in range(NQC):
                    for m in range(DCH):
                        ps = mm_ps.tile([P, 512], f32, tag="mm",
                                        name=f"wproj{qq}{m}")
                        for pair in range(4):
                            nc.tensor.matmul(
                                ps[:],
                                wo_sb[:, pair, m * P:(m + 1) * P],
                                o_pair[pair][:, qq * 512:(qq + 1) * 512],
                                start=(pair == 0), stop=(pair == 3),
                            )
                        ot = out_pool.tile([P, 512], f32, tag="out")
                        nc.vector.tensor_copy(ot[:], ps[:])
                        eng = nc.sync if m % 2 == 0 else nc.scalar
                        eng.dma_start(
                            outT[m * P:(m + 1) * P,
                                 qq * 512:(qq + 1) * 512], ot[:])
            else:
                # coarse: one q-group (+ next pair's k-group) per chunk
                # boundary; V features woven per-k-tile into pair-0 chunks
                # 0 (pairs 0-1 heads) and 1 (pairs 2-3 heads).  The final
                # projection runs COLUMN-WISE: the pass for q-columns qq
                # is emitted right after pair-3 chunk qq, so it overlaps
                # the remaining attention and only the last pass is tail.
                def proj_pass(qq):
                    for m in range(DCH):
                        ps = mm_ps.tile([P, 512], f32, tag="mm",
                                        name=f"proj{qq}{m}")
                        for pair in range(4):
                            nc.tensor.matmul(
                                ps[:],
                                wo_sb[:, pair, m * P:(m + 1) * P],
                                o_pair[pair][:, qq * 512:(qq + 1) * 512],
                                start=(pair == 0), stop=(pair == 3),
                            )
                        ot = out_pool.tile([P, 512], f32, tag="out")
                        nc.vector.tensor_copy(ot[:], ps[:])
                        eng = nc.sync if m % 2 == 0 else nc.scalar
                        eng.dma_start(
                            outT[m * P:(m + 1) * P,
                                 qq * 512:(qq + 1) * 512], ot[:])

                for pair in range(4):
                    for qq in range(NQC):
                        for _, fn in qk_group(pair, wqk[pair], 0, qq):
                            fn()
                        if pair < 3:
                            for _, fn in qk_group(pair + 1, wqk[pair + 1],
                                                  1, qq):
                                fn()
                        # fh0 V (pairs 0-1 heads) inline in chunk (0,0);
                        # fh1 V (pairs 2-3, deadline pair-2 c0) spread 4
                        # groups per chunk over (0,1)..(1,0)
                        vi = 0 if (pair == 0 and qq == 0) else None
                        ck = pair * 4 + qq
                        vl = (list(range((ck - 1) * 2, ck * 2))
                              if 1 <= ck <= 8 else None)
                        emit_attn_chunk(pair, qq, v_il=vi, v_list=vl)
                    if pair < 2:
                        sb_, dma_ = wqk_dma_fns(pair + 2)
                        wqk[pair + 2] = sb_
                        dma_()
                # projection as pure tail: in-loop emission stalls the
                # in-order PE stream on the denominator DMA round trip
                for qq in range(NQC):
                    proj_pass(qq)


def _to_bf16(a):
    return np.asarray(a, np.float32).astype(ml_dtypes.bfloat16)


def _prep_inputs(x, ln_gamma, ln_beta, W_qkv, W_out):
    """Build the 8 per-core input maps (host-side, cheap numpy)."""
    scale = HD ** -0.5
    Wg = (W_qkv * ln_gamma[None, :].astype(np.float32)).astype(np.float32)
    cfull = (W_qkv @ ln_beta.astype(np.float32)).astype(np.float32)
    in_maps = []
    for c in range(8):
        bi, hg = c // 2, c % 2
        r0 = hg * INNER_C
        wq = Wg[r0:r0 + INNER_C] * scale
        wk = Wg[1024 + r0:1024 + r0 + INNER_C]
        wv = Wg[2048 + r0:2048 + r0 + INNER_C]
        cq = cfull[r0:r0 + INNER_C] * scale
        ck = cfull[1024 + r0:1024 + r0 + INNER_C]
        cvv = cfull[2048 + r0:2048 + r0 + INNER_C]
        cqk = np.empty((P, 8), np.float32)
        for p in range(4):
            cqk[:, p] = cq[p * P:(p + 1) * P]
            cqk[:, 4 + p] = ck[p * P:(p + 1) * P]
        in_maps.append({
            "onesc": np.full((P, P), 1.0 / D, np.float32),
            "vones": np.ones((P, KT, 8, 1), ml_dtypes.bfloat16),
            "xT": np.ascontiguousarray(x[bi].T).astype(np.float32),
            "wqkT": _to_bf16(np.concatenate([wq, wk], 0).T),
            "wvT": _to_bf16(wv.T),
            "woT": _to_bf16(W_out[:, r0:r0 + INNER_C].T),
            "cqk": cqk,
            "cv": cvv.reshape(1, INNER_C),
        })
    return in_maps


_NC_CACHE = None


def kernel(x, ln_gamma, ln_beta, W_qkv, W_out):
    from concourse.bass_utils import run_bass_kernel_spmd
    global _NC_CACHE
    x = np.asarray(x, np.float32)
    in_maps = _prep_inputs(
        x, np.asarray(ln_gamma, np.float32), np.asarray(ln_beta, np.float32),
        np.asarray(W_qkv, np.float32), np.asarray(W_out, np.float32))
    if _NC_CACHE is None:
        _NC_CACHE = build_nc()
    res = run_bass_kernel_spmd(_NC_CACHE, in_maps, list(range(8))).results
    b, n, d = x.shape
    out = np.empty((b, n, d), np.float32)
    for bi in range(b):
        out[bi] = (res[2 * bi]["outT"] + res[2 * bi + 1]["outT"]).T
    return out
